# revision 10
# baseline (speedup 1.0000x reference)
"""Trainium2 Bass kernel: multi-head attention with sequence-axis layernorm
and relative position bias, sharded 8-way over heads (2 heads/core).

Layout strategy (all per core):
  - LN over sequence axis done in [d_partition, n_free] layout (xT input).
  - qT/kT produced transposed [inner_local=128, b*n] (head-dim on partitions)
    so sim can be computed TRANSPOSED: simT[nj, ni] = kT.T @ qT (K=dh=64).
  - bias is pre-transposed per head on the host, so bias-add happens in
    simT layout with a plain DVE tensor_tensor add.
  - softmax without max-subtraction (scores ~ N(0,2); exp is safe in f32);
    unnormalized attn exp'd on ScalarE; Z obtained by appending a ones
    column to v (M=65 in the av matmul); division folded in after av.
  - av matmul: lhsT = v_aug [nj,65], rhs = attn_uT [nj, ni] -> psum [65, ni]
    accumulated over nj tiles; no transposes anywhere on chip.
  - out projection: per-head K=64 matmuls accumulating both heads into one
    PSUM group; each core emits a partial [B*N, DIM] summed on host.
"""

import numpy as np
import ml_dtypes

import concourse.bass as bass
from concourse import bacc
import concourse.mybir as mybir
import concourse.tile as tile
from concourse.bass_utils import run_bass_kernel_spmd

F32 = mybir.dt.float32
BF16 = mybir.dt.bfloat16
BF = ml_dtypes.bfloat16
AF = mybir.ActivationFunctionType
ALU = mybir.AluOpType

# full-size problem constants
B, N, DIM = 2, 2048, 1024
HEADS, DH = 16, 64
NCORES = 8
HL = HEADS // NCORES          # heads per core = 2
IL = HL * DH                  # local inner = 128
INNER = HEADS * DH            # 1024


def build(b_sz=B, n_sz=N, dim=DIM, eps=1e-5):
    """Build the per-core Bass graph (SPMD across 8 cores)."""
    nd = dim // 128               # d tiles
    nch = (b_sz * n_sz) // 512    # 512-col chunks of flattened b*n
    njb = n_sz // 128             # key tiles per batch
    nic = n_sz // 512             # query chunks per batch
    bn = b_sz * n_sz
    nsub = n_sz // 512            # bn_stats subgroups

    nc = bacc.Bacc(None, target_bir_lowering=False)
    xT = nc.declare_dram_parameter("xT", [b_sz, dim, n_sz], F32, isOutput=False)
    wqT = nc.declare_dram_parameter("wqT", [dim, IL], BF16, isOutput=False)
    wkT = nc.declare_dram_parameter("wkT", [dim, IL], BF16, isOutput=False)
    wvT = nc.declare_dram_parameter("wvT", [dim, IL], BF16, isOutput=False)
    woT = nc.declare_dram_parameter("woT", [IL, dim], BF16, isOutput=False)
    biasT = nc.declare_dram_parameter("biasT", [HL, n_sz, n_sz], BF16, isOutput=False)
    out = nc.declare_dram_parameter("out", [bn, dim], F32, isOutput=True)
    zdram = nc.dram_tensor("zscratch", [b_sz, HL, n_sz // 512, 1, 512], F32)

    with tile.TileContext(nc) as tc:
        with (
            tc.tile_pool(name="consts", bufs=1) as consts,
            tc.tile_pool(name="persist", bufs=1) as persist,
        ):
            # ---- load weights / g ----
            wq_s, wk_s, wv_s = [], [], []
            for dt in range(nd):
                for lst, src, nm in ((wq_s, wqT, "wq"), (wk_s, wkT, "wk"), (wv_s, wvT, "wv")):
                    t = consts.tile([128, IL], BF16, tag=f"{nm}{dt}")
                    nc.sync.dma_start(out=t, in_=src[dt * 128:(dt + 1) * 128, :])
                    lst.append(t)

            wo_h = []
            for h in range(HL):
                t = consts.tile([DH, dim], BF16, tag=f"wo{h}")
                nc.sync.dma_start(out=t, in_=woT[h * DH:(h + 1) * DH, :])
                wo_h.append(t)

            xn = {}
            qT = persist.tile([IL, bn], BF16, tag="qT")
            kT = persist.tile([IL, bn], BF16, tag="kT")
            va = {}
            avn = {}

            # ---- Phase 1: layernorm over sequence axis ----
            with (
                tc.tile_pool(name="xload", bufs=2) as xload,
                tc.tile_pool(name="lns", bufs=4) as lns,
            ):
                for b in range(b_sz):
                    for dt in range(nd):
                        xt = xload.tile([128, n_sz], F32, tag="xt")
                        nc.sync.dma_start(out=xt, in_=xT[b, dt * 128:(dt + 1) * 128, :])
                        stats = lns.tile([128, nsub, 6], F32, tag="stats")
                        for s in range(nsub):
                            nc.vector.bn_stats(out=stats[:, s, :], in_=xt[:, s * 512:(s + 1) * 512])
                        mv = lns.tile([128, 2], F32, tag="mv")
                        nc.vector.bn_aggr(out=mv, in_=stats)
                        vmax = lns.tile([128, 1], F32, tag="vmax")
                        nc.vector.tensor_scalar_max(vmax, mv[:, 1:2], eps)
                        sq = lns.tile([128, 1], F32, tag="sq")
                        nc.scalar.activation(out=sq, in_=vmax, func=AF.Sqrt)
                        scl = lns.tile([128, 1], F32, tag="scl")
                        nc.vector.reciprocal(scl, sq)
                        shf = lns.tile([128, 1], F32, tag="shf")
                        nc.vector.tensor_mul(shf, mv[:, 0:1], scl)
                        xnt = persist.tile([128, n_sz], BF16, tag=f"xn_{b}_{dt}")
                        nc.vector.tensor_scalar(
                            out=xnt, in0=xt, scalar1=scl, scalar2=shf,
                            op0=ALU.mult, op1=ALU.subtract,
                        )
                        xn[b, dt] = xnt

            # ---- Phase 2a: q/k projections (transposed layout) ----
            with tc.tile_pool(name="pproj", bufs=4, space="PSUM") as pproj:
                for ch in range(nch):
                    b = (ch * 512) // n_sz
                    col0 = (ch * 512) % n_sz
                    for (w_s, dst) in ((wq_s, qT), (wk_s, kT)):
                        ps = pproj.tile([IL, 512], F32, tag="ps")
                        for dt in range(nd):
                            nc.tensor.matmul(
                                ps, w_s[dt], xn[b, dt][:, col0:col0 + 512],
                                start=(dt == 0), stop=(dt == nd - 1),
                            )
                        nc.vector.tensor_copy(dst[:, ch * 512:(ch + 1) * 512], ps)

            # ---- Phase 2b: v natural + ones column ----
            with tc.tile_pool(name="pv", bufs=4, space="PSUM") as pv:
                for b in range(b_sz):
                    for nj in range(njb):
                        psv = pv.tile([128, IL], F32, tag="psv")
                        for dt in range(nd):
                            nc.tensor.matmul(
                                psv, xn[b, dt][:, nj * 128:(nj + 1) * 128], wv_s[dt],
                                start=(dt == 0), stop=(dt == nd - 1),
                            )
                        for h in range(HL):
                            t = persist.tile([128, DH + 1], BF16, tag=f"va_{b}_{h}_{nj}")
                            nc.vector.tensor_copy(t[:, 0:DH], psv[:, h * DH:(h + 1) * DH])
                            nc.vector.memset(t[:, DH:DH + 1], 1.0)
                            va[b, h, nj] = t

            # ---- Phase 3: attention per (b, local head) ----
            for b in range(b_sz):
                for h in range(HL):
                    kTh = kT[h * DH:(h + 1) * DH, b * n_sz:(b + 1) * n_sz]
                    qTh = qT[h * DH:(h + 1) * DH, b * n_sz:(b + 1) * n_sz]
                    avh = persist.tile([DH, n_sz], BF16, tag=f"avn_{b}_{h}")
                    avn[b, h] = avh
                    with (
                        tc.tile_pool(name=f"psim_{b}_{h}", bufs=4, space="PSUM") as psim,
                        tc.tile_pool(name=f"pav_{b}_{h}", bufs=1, space="PSUM") as pavp,
                        tc.tile_pool(name=f"attn_{b}_{h}", bufs=6) as attnp,
                        tc.tile_pool(name=f"bias_{b}_{h}", bufs=6) as biasp,
                        tc.tile_pool(name=f"z_{b}_{h}", bufs=2) as zp,
                    ):
                        pavs = [
                            pavp.tile([DH + 1, 512], F32, tag=f"pav{ni}", name=f"pav_{b}_{h}_{ni}")
                            for ni in range(nic)
                        ]
                        for nj in range(njb):
                            sims = []
                            for ni in range(nic):
                                ps = psim.tile([128, 512], F32, tag="ps")
                                nc.tensor.matmul(
                                    ps, kTh[:, nj * 128:(nj + 1) * 128],
                                    qTh[:, ni * 512:(ni + 1) * 512],
                                    start=True, stop=True,
                                )
                                sims.append(ps)
                            for ni in range(nic):
                                ps = sims[ni]
                                bt = biasp.tile([128, 512], BF16, tag="bt")
                                nc.sync.dma_start(
                                    out=bt,
                                    in_=biasT[h, nj * 128:(nj + 1) * 128, ni * 512:(ni + 1) * 512],
                                )
                                nc.vector.tensor_add(ps, ps, bt)
                                ae = attnp.tile([128, 512], BF16, tag="ae")
                                nc.scalar.activation(out=ae, in_=ps, func=AF.Exp)
                                nc.tensor.matmul(
                                    pavs[ni], va[b, h, nj], ae,
                                    start=(nj == 0), stop=(nj == njb - 1),
                                )
                        for ni in range(nic):
                            zf = zp.tile([DH + 1, 512], F32, tag="zf")
                            nc.vector.reciprocal(zf[DH:DH + 1, :], pavs[ni][DH:DH + 1, :])
                            nc.sync.dma_start(out=zdram[b, h, ni], in_=zf[DH:DH + 1, :])
                            zb = zp.tile([DH, 512], F32, tag="zb")
                            nc.sync.dma_start(out=zb, in_=zdram[b, h, ni].to_broadcast([DH, 512]))
                            nc.vector.tensor_mul(
                                avh[:, ni * 512:(ni + 1) * 512], pavs[ni][0:DH, :], zb,
                            )

            # ---- Phase 4: output projection (partial over local heads) ----
            with (
                tc.tile_pool(name="pout", bufs=2, space="PSUM") as pout,
                tc.tile_pool(name="ost", bufs=3) as ost,
            ):
                for blk in range(bn // 128):
                    b = (blk * 128) // n_sz
                    r0 = (blk * 128) % n_sz
                    po = pout.tile([128, dim], F32, tag="po")
                    for c0 in range(0, dim, 512):
                        w = min(512, dim - c0)
                        for h in range(HL):
                            nc.tensor.matmul(
                                po[:, c0:c0 + w],
                                avn[b, h][:, r0:r0 + 128],
                                wo_h[h][:, c0:c0 + w],
                                start=(h == 0), stop=(h == HL - 1),
                            )
                    os_ = ost.tile([128, dim], F32, tag="os")
                    nc.vector.tensor_copy(os_, po)
                    nc.sync.dma_start(out=out[blk * 128:(blk + 1) * 128, :], in_=os_)
    nc.compile()
    return nc


_NC_CACHE = {}


def _get_nc(key, **kw):
    if key not in _NC_CACHE:
        _NC_CACHE[key] = build(**kw)
    return _NC_CACHE[key]


def make_in_maps(x, rel_pos_bias, g, Wq, Wkv, Wo):
    b_sz, n_sz, dim = x.shape
    inner = Wq.shape[0]
    x = np.asarray(x, np.float32)
    xTh = np.ascontiguousarray(x.transpose(0, 2, 1))          # [B, DIM, N]
    gv = np.asarray(g, np.float32).reshape(1, dim)
    Wq = np.asarray(Wq, np.float32) * gv
    Wkv = np.asarray(Wkv, np.float32) * gv
    scale = DH ** -0.5
    in_maps = []
    for c in range(NCORES):
        rs, re = c * IL, (c + 1) * IL
        wq_c = np.ascontiguousarray((np.asarray(Wq)[rs:re, :] * scale).T).astype(BF)
        wk_c = np.ascontiguousarray(np.asarray(Wkv)[rs:re, :].T).astype(BF)
        wv_c = np.ascontiguousarray(np.asarray(Wkv)[inner + rs:inner + re, :].T).astype(BF)
        wo_c = np.ascontiguousarray(np.asarray(Wo)[:, rs:re].T).astype(BF)
        bias_c = np.ascontiguousarray(
            np.asarray(rel_pos_bias)[0, c * HL:(c + 1) * HL].transpose(0, 2, 1)
        ).astype(BF)
        in_maps.append({
            "xT": xTh, "wqT": wq_c, "wkT": wk_c, "wvT": wv_c,
            "woT": wo_c, "biasT": bias_c,
        })
    return in_maps


def kernel(x, rel_pos_bias, g, Wq, Wkv, Wo):
    b_sz, n_sz, dim = x.shape
    nc = _get_nc((b_sz, n_sz, dim), b_sz=b_sz, n_sz=n_sz, dim=dim)
    in_maps = make_in_maps(x, rel_pos_bias, g, Wq, Wkv, Wo)
    res = run_bass_kernel_spmd(nc, in_maps, core_ids=list(range(NCORES)))
    acc = np.zeros((b_sz * n_sz, dim), np.float32)
    for r in res.results:
        acc += np.asarray(r["out"], np.float32)
    return np.ascontiguousarray(acc.reshape(b_sz, n_sz, dim))


# revision 11
# speedup vs baseline: 1.1166x; 1.1166x over previous
"""Trainium2 Bass kernel: multi-head attention with sequence-axis layernorm
and relative position bias, sharded 8-way over heads (2 heads/core).

Layout strategy (all per core):
  - LN over sequence axis done in [d_partition, n_free] layout (xT input);
    g is folded into Wq/Wkv on the host.
  - qT/kT produced transposed [inner_local=128, b*n] (head-dim on partitions)
    so sim is computed TRANSPOSED: simT[nj, ni] = kT.T @ qT (K=dh=64), with
    the two local heads row-tiled into the PE array concurrently (rows 0-63 /
    64-127 via base_partition auto tile_position).
  - bias (host-pretransposed per head, bf16) is seeded into PSUM with an
    identity matmul (start=True); the sim matmul accumulates on top
    (start=False). No DVE pass for the bias add.
  - softmax without max-subtraction (scores ~ N(0,2); exp safe in f32);
    ScalarE exp reads PSUM [128, 1024] spans directly, writes bf16 attn_uT.
  - av matmul: lhsT = v_aug [nj, 65] (ones column -> row 64 = Z), rhs =
    attn_uT, accumulated over nj into [65, 512] PSUM chunks.
  - normalization by 1/Z is folded into the OUTPUT projection: per-head
    PSUM partials scaled by per-partition 1/Z columns (Z round-trips
    through DRAM to transpose rows->columns).
"""

import numpy as np
import ml_dtypes

import concourse.bass as bass
from concourse import bacc
import concourse.mybir as mybir
import concourse.tile as tile
from concourse.masks import make_identity
from concourse.bass_utils import run_bass_kernel_spmd

F32 = mybir.dt.float32
BF16 = mybir.dt.bfloat16
BF = ml_dtypes.bfloat16
AF = mybir.ActivationFunctionType
ALU = mybir.AluOpType

# full-size problem constants
B, N, DIM = 2, 2048, 1024
HEADS, DH = 16, 64
NCORES = 8
HL = HEADS // NCORES          # heads per core = 2
IL = HL * DH                  # local inner = 128
INNER = HEADS * DH            # 1024


def build(b_sz=B, n_sz=N, dim=DIM, eps=1e-5):
    """Build the per-core Bass graph (SPMD across 8 cores)."""
    nd = dim // 128               # d tiles
    nch = (b_sz * n_sz) // 512    # 512-col chunks of flattened b*n
    njb = n_sz // 128             # key tiles per batch
    nic = n_sz // 512             # query chunks per batch
    bn = b_sz * n_sz
    nsub = n_sz // 512            # bn_stats subgroups

    nc = bacc.Bacc(None, target_bir_lowering=False)
    xT = nc.declare_dram_parameter("xT", [b_sz, dim, n_sz], F32, isOutput=False)
    wqT = nc.declare_dram_parameter("wqT", [dim, IL], BF16, isOutput=False)
    wkT = nc.declare_dram_parameter("wkT", [dim, IL], BF16, isOutput=False)
    wvT = nc.declare_dram_parameter("wvT", [dim, IL], BF16, isOutput=False)
    woT = nc.declare_dram_parameter("woT", [IL, dim], BF16, isOutput=False)
    biasT = nc.declare_dram_parameter("biasT", [HL, n_sz, n_sz], BF16, isOutput=False)
    out = nc.declare_dram_parameter("out", [bn, dim], F32, isOutput=True)
    zdram = nc.dram_tensor("zscratch", [b_sz, HL, n_sz], BF16)

    with tile.TileContext(nc) as tc:
        with (
            tc.tile_pool(name="consts", bufs=1) as consts,
            tc.tile_pool(name="persist", bufs=1) as persist,
        ):
            # ---- load weights; build identity ----
            wq_s, wk_s, wv_s = [], [], []
            for dt in range(nd):
                for lst, src, nm in ((wq_s, wqT, "wq"), (wk_s, wkT, "wk"), (wv_s, wvT, "wv")):
                    t = consts.tile([128, IL], BF16, tag=f"{nm}{dt}")
                    nc.sync.dma_start(out=t, in_=src[dt * 128:(dt + 1) * 128, :])
                    lst.append(t)
            wo_h = []
            for h in range(HL):
                t = consts.tile([DH, dim], BF16, tag=f"wo{h}")
                nc.sync.dma_start(out=t, in_=woT[h * DH:(h + 1) * DH, :])
                wo_h.append(t)
            ident = consts.tile([128, 128], BF16, tag="ident")
            make_identity(nc, ident)

            xn = {}
            qT = persist.tile([IL, bn], BF16, tag="qT")
            kT = persist.tile([IL, bn], BF16, tag="kT")
            va = {}
            avz = {}   # (b, h) -> [DH+1, n] bf16, rows 0..63 = av_u, row 64 = Z

            # ---- Phase 1: layernorm over sequence axis ----
            with (
                tc.tile_pool(name="xload", bufs=2) as xload,
                tc.tile_pool(name="lns", bufs=4) as lns,
            ):
                for b in range(b_sz):
                    for dt in range(nd):
                        xt = xload.tile([128, n_sz], F32, tag="xt")
                        nc.sync.dma_start(out=xt, in_=xT[b, dt * 128:(dt + 1) * 128, :])
                        stats = lns.tile([128, nsub, 6], F32, tag="stats")
                        for s in range(nsub):
                            nc.vector.bn_stats(out=stats[:, s, :], in_=xt[:, s * 512:(s + 1) * 512])
                        mv = lns.tile([128, 2], F32, tag="mv")
                        nc.vector.bn_aggr(out=mv, in_=stats)
                        vmax = lns.tile([128, 1], F32, tag="vmax")
                        nc.vector.tensor_scalar_max(vmax, mv[:, 1:2], eps)
                        sq = lns.tile([128, 1], F32, tag="sq")
                        nc.scalar.activation(out=sq, in_=vmax, func=AF.Sqrt)
                        scl = lns.tile([128, 1], F32, tag="scl")
                        nc.vector.reciprocal(scl, sq)
                        shf = lns.tile([128, 1], F32, tag="shf")
                        nc.vector.tensor_mul(shf, mv[:, 0:1], scl)
                        xnt = persist.tile([128, n_sz], BF16, tag=f"xn_{b}_{dt}")
                        nc.vector.tensor_scalar(
                            out=xnt, in0=xt, scalar1=scl, scalar2=shf,
                            op0=ALU.mult, op1=ALU.subtract,
                        )
                        xn[b, dt] = xnt

            # ---- Phase 2a: q/k projections (transposed layout) ----
            with tc.tile_pool(name="pproj", bufs=4, space="PSUM") as pproj:
                for ch in range(nch):
                    b = (ch * 512) // n_sz
                    col0 = (ch * 512) % n_sz
                    for (w_s, dst) in ((wq_s, qT), (wk_s, kT)):
                        ps = pproj.tile([IL, 512], F32, tag="ps")
                        for dt in range(nd):
                            nc.tensor.matmul(
                                ps, w_s[dt], xn[b, dt][:, col0:col0 + 512],
                                start=(dt == 0), stop=(dt == nd - 1),
                            )
                        nc.vector.tensor_copy(dst[:, ch * 512:(ch + 1) * 512], ps)

            # ---- Phase 2b: v natural + ones column ----
            with tc.tile_pool(name="pv", bufs=4, space="PSUM") as pv:
                for b in range(b_sz):
                    for nj in range(njb):
                        psv = pv.tile([128, IL], F32, tag="psv")
                        for dt in range(nd):
                            nc.tensor.matmul(
                                psv, xn[b, dt][:, nj * 128:(nj + 1) * 128], wv_s[dt],
                                start=(dt == 0), stop=(dt == nd - 1),
                            )
                        for h in range(HL):
                            t = persist.tile([128, DH + 1], BF16, tag=f"va_{b}_{h}_{nj}")
                            nc.vector.tensor_copy(t[:, 0:DH], psv[:, h * DH:(h + 1) * DH])
                            nc.vector.memset(t[:, DH:DH + 1], 1.0)
                            va[b, h, nj] = t

            # ---- Phase 3: attention per (b, ni-group of up to 2 chunks) ----
            for b in range(b_sz):
                for h in range(HL):
                    avz[b, h] = persist.tile([DH + 1, n_sz], BF16, tag=f"avz_{b}_{h}",
                                             name=f"avz_{b}_{h}")
            for b in range(b_sz):
                kTb = kT[:, b * n_sz:(b + 1) * n_sz]
                qTb = qT[:, b * n_sz:(b + 1) * n_sz]
                for g0 in range(0, nic, 2):
                    gn = min(2, nic - g0)
                    W = gn * 512
                    with (
                        tc.tile_pool(name=f"psim_{b}_{g0}", bufs=1, space="PSUM") as psim,
                        tc.tile_pool(name=f"pav_{b}_{g0}", bufs=1, space="PSUM") as pavp,
                        tc.tile_pool(name=f"attn_{b}_{g0}", bufs=2) as attnp,
                        tc.tile_pool(name=f"bias_{b}_{g0}", bufs=6) as biasp,
                    ):
                        pavs = {}
                        for h in range(HL):
                            for gi in range(gn):
                                pavs[h, gi] = pavp.tile(
                                    [DH + 1, 512], F32, tag=f"pav{h}_{gi}",
                                    name=f"pav_{b}_{g0}_{h}_{gi}")
                        for nj in range(njb):
                            pst = {}
                            # seed PSUM with bias via identity matmul
                            for h in range(HL):
                                pst[h] = psim.tile([128, W], F32, tag=f"ps{h}",
                                                   name=f"ps_{b}_{g0}_{h}_{nj}")
                                for gi in range(gn):
                                    bt = biasp.tile([128, 512], BF16, tag="bt", name="bt")
                                    nc.sync.dma_start(
                                        out=bt,
                                        in_=biasT[h, nj * 128:(nj + 1) * 128,
                                                  (g0 + gi) * 512:(g0 + gi + 1) * 512],
                                    )
                                    nc.tensor.matmul(
                                        pst[h][:, gi * 512:(gi + 1) * 512], ident, bt,
                                        start=True, stop=False,
                                    )
                            # paired sim matmuls (h0 rows 0-63, h1 rows 64-127)
                            for h in range(HL):
                                kTh = kTb[h * DH:(h + 1) * DH, nj * 128:(nj + 1) * 128]
                                for gi in range(gn):
                                    nc.tensor.matmul(
                                        pst[h][:, gi * 512:(gi + 1) * 512],
                                        kTh,
                                        qTb[h * DH:(h + 1) * DH,
                                            (g0 + gi) * 512:(g0 + gi + 1) * 512],
                                        start=False, stop=True,
                                    )
                            # exp over the whole [128, W] span, PSUM -> SBUF bf16
                            aes = {}
                            for h in range(HL):
                                ae = attnp.tile([128, W], BF16, tag=f"ae{h}", name="ae")
                                nc.scalar.activation(out=ae, in_=pst[h], func=AF.Exp)
                                aes[h] = ae
                            # av accumulate (M=65 incl. ones column -> Z in row 64)
                            for h in range(HL):
                                for gi in range(gn):
                                    nc.tensor.matmul(
                                        pavs[h, gi], va[b, h, nj],
                                        aes[h][:, gi * 512:(gi + 1) * 512],
                                        start=(nj == 0), stop=(nj == njb - 1),
                                    )
                        # evacuate av_u + Z to SBUF (bf16)
                        for h in range(HL):
                            for gi in range(gn):
                                nc.vector.tensor_copy(
                                    avz[b, h][:, (g0 + gi) * 512:(g0 + gi + 1) * 512],
                                    pavs[h, gi],
                                )
            # Z rows -> DRAM (to reload as per-partition columns)
            for b in range(b_sz):
                for h in range(HL):
                    nc.sync.dma_start(out=zdram[b, h, :], in_=avz[b, h][DH:DH + 1, :])

            # ---- Phase 4: output projection with fused 1/Z normalization ----
            with (
                tc.tile_pool(name="pout", bufs=2, space="PSUM") as pout,
                tc.tile_pool(name="ost", bufs=3) as ost,
                tc.tile_pool(name="zc", bufs=1) as zc,
            ):
                zrec = {}
                for b in range(b_sz):
                    zcol = zc.tile([128, HL, njb], BF16, tag=f"zc{b}", name=f"zc{b}")
                    nc.sync.dma_start(
                        out=zcol,
                        in_=zdram[b].rearrange("h (c p) -> p h c", p=128),
                    )
                    zr = zc.tile([128, HL, njb], F32, tag=f"zr{b}", name=f"zr{b}")
                    nc.vector.reciprocal(zr, zcol)
                    zrec[b] = zr
                for blk in range(bn // 128):
                    b = (blk * 128) // n_sz
                    r0 = (blk * 128) % n_sz
                    jb = r0 // 128
                    po = {}
                    for h in range(HL):
                        po[h] = pout.tile([128, dim], F32, tag=f"po{h}", name=f"po{h}")
                        for c0 in range(0, dim, 512):
                            w = min(512, dim - c0)
                            nc.tensor.matmul(
                                po[h][:, c0:c0 + w],
                                avz[b, h][0:DH, r0:r0 + 128],
                                wo_h[h][:, c0:c0 + w],
                                start=True, stop=True,
                            )
                    os_ = ost.tile([128, dim], F32, tag="os", name="os")
                    nc.vector.tensor_scalar_mul(os_, po[0], zrec[b][:, 0, jb:jb + 1])
                    nc.vector.scalar_tensor_tensor(
                        out=os_, in0=po[1], scalar=zrec[b][:, 1, jb:jb + 1],
                        in1=os_, op0=ALU.mult, op1=ALU.add,
                    )
                    nc.sync.dma_start(out=out[blk * 128:(blk + 1) * 128, :], in_=os_)
    nc.compile()
    return nc


_NC_CACHE = {}


def _get_nc(key, **kw):
    if key not in _NC_CACHE:
        _NC_CACHE[key] = build(**kw)
    return _NC_CACHE[key]


def make_in_maps(x, rel_pos_bias, g, Wq, Wkv, Wo):
    b_sz, n_sz, dim = x.shape
    inner = Wq.shape[0]
    x = np.asarray(x, np.float32)
    xTh = np.ascontiguousarray(x.transpose(0, 2, 1))          # [B, DIM, N]
    gv = np.asarray(g, np.float32).reshape(1, dim)
    Wq = np.asarray(Wq, np.float32) * gv
    Wkv = np.asarray(Wkv, np.float32) * gv
    scale = DH ** -0.5
    in_maps = []
    for c in range(NCORES):
        rs, re = c * IL, (c + 1) * IL
        wq_c = np.ascontiguousarray((Wq[rs:re, :] * scale).T).astype(BF)
        wk_c = np.ascontiguousarray(Wkv[rs:re, :].T).astype(BF)
        wv_c = np.ascontiguousarray(Wkv[inner + rs:inner + re, :].T).astype(BF)
        wo_c = np.ascontiguousarray(np.asarray(Wo)[:, rs:re].T).astype(BF)
        bias_c = np.ascontiguousarray(
            np.asarray(rel_pos_bias)[0, c * HL:(c + 1) * HL].transpose(0, 2, 1)
        ).astype(BF)
        in_maps.append({
            "xT": xTh, "wqT": wq_c, "wkT": wk_c, "wvT": wv_c,
            "woT": wo_c, "biasT": bias_c,
        })
    return in_maps


def kernel(x, rel_pos_bias, g, Wq, Wkv, Wo):
    b_sz, n_sz, dim = x.shape
    nc = _get_nc((b_sz, n_sz, dim), b_sz=b_sz, n_sz=n_sz, dim=dim)
    in_maps = make_in_maps(x, rel_pos_bias, g, Wq, Wkv, Wo)
    res = run_bass_kernel_spmd(nc, in_maps, core_ids=list(range(NCORES)))
    acc = np.zeros((b_sz * n_sz, dim), np.float32)
    for r in res.results:
        acc += np.asarray(r["out"], np.float32)
    return np.ascontiguousarray(acc.reshape(b_sz, n_sz, dim))
